# revision 4
# baseline (speedup 1.0000x reference)
"""AdaFace loss on 8 TRN2 NeuronCores, class-parallel.

Strategy: shard the 100k weight rows (classes) across 8 cores. Host
pre-normalizes rows, transposes to [D, C_shard], scales by 8 and casts to
fp8e4; device computes 64*cos via fp8 DoubleRow matmuls and the
softmax denominator sum(exp(32cos-32)) per batch row. Fixed shift 32
replaces the row max (|logit|<=32), so no collective. Host does the
O(B) combine: margin-target correction, ln, weighted dot.

Device pipeline (v3): PSUM is the choke point (only ACT and DVE can
read it, ~1.1-1.15 ns/col each). PSUM is split into 4 independent
1024-col slots (2 banks each, bufs=1 tags), tiles strictly alternate
ACT/DVE so each engine owns a double-buffered pair of slots and the
PE's production (474ns/tile) hides under consumption (1.2-1.4us/tile):
 - ACT: exp activation + fused accum_out row-sum per tile.
 - DVE: Schraudolph fake-exp (affine to i16 bits = bf16(exp)) into a
   6-plane fi tile; Pool folds planes with two 2048-wide bf16 adds;
   DVE does one 2048-wide row-reduce per batch chunk.
All partial sums land in one [128, NBC, NSLOT] f32 tile, DMA'd out
once; the host sums slots and finishes the loss.
"""

import numpy as np
import ml_dtypes

import concourse.bass as bass
import concourse.tile as tile
from concourse import bacc, mybir
from concourse.bass_utils import run_bass_kernel_spmd

B = 512
D = 256
C = 100000
NCORES = 8
CSH = C // NCORES          # 12500 classes per core
CPAD = 12544               # 12*1024 + 256
NPAD_TOT = (CPAD - CSH) * NCORES

M0 = 0.5
M_MIN = 0.25
SCALE = 32.0
SHIFT = 32.0
FP8_PRESCALE = 8.0         # both operands scaled by 8 -> matmul gives 64*cos

LOG2E = 1.4426950408889634
FA = 64.0 * LOG2E
FB = 16256.0 - 4096.0 * LOG2E

f32 = mybir.dt.float32
bf16 = mybir.dt.bfloat16
i16 = mybir.dt.int16
fp8 = mybir.dt.float8e4

NBC = B // 128             # 4 batch chunks
NSLOT = 10                 # 7 ACT accum slots (6 wide + small) + 2 DVE

_cached_nc = None
_last_results = None


def _schraudolph_rho(fb):
    t = np.linspace(-60.0, -1.0, 200001)
    x = (t + 32.0) * 2.0
    y = np.float32(x) * np.float32(FA) + np.float32(fb)
    i = np.rint(y).astype(np.int16)
    v = i.view(ml_dtypes.bfloat16).astype(np.float64)
    return float(np.mean(v / np.exp(t)))


FB_EFF = FB - 128.0 * np.log2(_schraudolph_rho(FB))
FB_EFF = FB_EFF - 128.0 * np.log2(_schraudolph_rho(FB_EFF))


def _build():
    global _cached_nc
    if _cached_nc is not None:
        return _cached_nc

    nc = bacc.Bacc(
        "TRN2", target_bir_lowering=False, debug=False, num_devices=NCORES
    )

    wnT_d = nc.dram_tensor("wnT", [128, 2, CPAD], fp8, kind="ExternalInput")
    featnT_d = nc.dram_tensor("featnT", [128, 2, B], fp8, kind="ExternalInput")
    out_d = nc.dram_tensor("out", [128, NBC, NSLOT], f32, kind="ExternalOutput")

    with tile.TileContext(nc) as tc:
        with (
            tc.tile_pool(name="persist", bufs=1) as persist,
            tc.tile_pool(name="work", bufs=2) as work,
            tc.tile_pool(name="psum", bufs=1, space="PSUM") as psum,
        ):
            fsb = persist.tile([128, 2, B], fp8)
            nc.sync.dma_start(out=fsb[:], in_=featnT_d[:])

            wsb = persist.tile([128, 2, CPAD], fp8)
            # weight loads in consumption order, 2 HWDGE queues, few sems
            plan = [
                (nc.scalar, 0, 2048),
                (nc.sync, 2048, 5120),
                (nc.scalar, 5120, 8192),
                (nc.sync, 8192, 12544),
            ]
            for eng, lo, hi in plan:
                eng.dma_start(out=wsb[:, :, lo:hi], in_=wnT_d[:, :, lo:hi])

            bias_s = persist.tile([128, 1], f32)
            nc.gpsimd.memset(bias_s[:], -SHIFT)

            S_out = persist.tile([128, NBC, NSLOT], f32)
            nc.gpsimd.memset(S_out[:], 0.0)

            esc = persist.tile([128, 1024], bf16)   # ACT dead-store target

            for bc in range(NBC):
                lhs = fsb[:, :, bc * 128:(bc + 1) * 128]
                fi6 = work.tile([128, 6, 1024], i16, tag="fi6")
                tacc = work.tile([128, 2, 1024], bf16, tag="tacc")
                a_slot = 0
                d_cnt = 0
                for ti in range(12):
                    c0 = ti * 1024
                    is_dve = (ti % 2 == 1)
                    ps = psum.tile([128, 1024], f32, tag=f"p{ti % 4}")
                    for j in (0, 512):
                        nc.tensor.matmul(
                            ps[:, j:j + 512],
                            lhs,
                            wsb[:, :, c0 + j:c0 + j + 512],
                            start=True, stop=True,
                            perf_mode=mybir.MatmulPerfMode.DoubleRow,
                        )
                    if not is_dve:
                        nc.scalar.activation(
                            esc[:], ps[:],
                            mybir.ActivationFunctionType.Exp,
                            bias=bias_s[:], scale=SCALE / (FP8_PRESCALE**2),
                            accum_out=S_out[:, bc, a_slot:a_slot + 1],
                        )
                        a_slot += 1
                    else:
                        nc.vector.tensor_scalar(
                            fi6[:, d_cnt, :], ps[:],
                            FA, FB_EFF,
                            mybir.AluOpType.mult, mybir.AluOpType.add,
                        )
                        d_cnt += 1
                        if d_cnt == 4:
                            # planes 0|1 + planes 2|3, 2048 wide, on Pool
                            nc.gpsimd.tensor_add(
                                tacc[:],
                                fi6[:, 0:2, :].bitcast(bf16),
                                fi6[:, 2:4, :].bitcast(bf16),
                            )

                # small 256-col tail tile -> ACT (slot p0 again)
                ps = psum.tile([128, 1024], f32, tag="p0")
                nc.tensor.matmul(
                    ps[:, 0:256],
                    lhs,
                    wsb[:, :, 12288:12544],
                    start=True, stop=True,
                    perf_mode=mybir.MatmulPerfMode.DoubleRow,
                )
                nc.scalar.activation(
                    esc[:, 0:256], ps[:, 0:256],
                    mybir.ActivationFunctionType.Exp,
                    bias=bias_s[:], scale=SCALE / (FP8_PRESCALE**2),
                    accum_out=S_out[:, bc, a_slot:a_slot + 1],
                )

                # fold planes 4|5 into tacc (Pool), then row-reduce (DVE)
                nc.gpsimd.tensor_add(
                    tacc[:], tacc[:],
                    fi6[:, 4:6, :].bitcast(bf16),
                )
                nc.vector.tensor_reduce(
                    S_out[:, bc, 8:10],
                    tacc[:],
                    axis=mybir.AxisListType.X,
                    op=mybir.AluOpType.add,
                )

            nc.sync.dma_start(out=out_d[:], in_=S_out[:])

    nc.compile()
    _cached_nc = nc
    return nc


def _host_prep(features, weight, weights, labels):
    f = features.astype(np.float64)
    norms = np.sqrt((f * f).sum(axis=1))
    lo, hi = norms.min(), norms.max()
    denom = max(hi - lo, 1e-8)
    margins = np.clip(M_MIN + (M0 - M_MIN) * (norms - lo) / denom, M_MIN, M0)
    feat_n = f / np.maximum(norms, 1e-12)[:, None]

    wlab = weight[labels].astype(np.float64)
    wlab_n = wlab / np.maximum(
        np.sqrt((wlab * wlab).sum(axis=1)), 1e-12
    )[:, None]
    cos_t = np.clip((feat_n * wlab_n).sum(axis=1), -1.0 + 1e-7, 1.0 - 1e-7)
    cos_m = cos_t * np.cos(margins) - np.sqrt(1.0 - cos_t * cos_t) * np.sin(
        margins
    )
    t_logit = SCALE * cos_m
    corr = (
        np.exp(SCALE * cos_m - SHIFT)
        - np.exp(SCALE * cos_t - SHIFT)
        - NPAD_TOT * np.exp(-SHIFT)
    )
    coef = weights.astype(np.float64) / B
    return feat_n, corr, coef, t_logit


def _to_dr_layout(mat_t, width):
    """[D, X] f32 -> [128, 2, X] fp8 with k = j*128 + p."""
    a = mat_t.reshape(2, 128, width)          # [j, p, X]
    a = np.ascontiguousarray(a.transpose(1, 0, 2))  # [p, j, X]
    return a.astype(ml_dtypes.float8_e4m3)


def kernel(features, weight, weights, labels):
    global _last_results
    features = np.asarray(features, dtype=np.float32)
    weight = np.asarray(weight, dtype=np.float32)
    weights = np.asarray(weights, dtype=np.float32)
    labels = np.asarray(labels).astype(np.int64)

    feat_n, corr, coef, t_logit = _host_prep(features, weight, weights, labels)

    wn = weight / np.maximum(
        np.linalg.norm(weight, axis=1, keepdims=True), 1e-12
    )
    featnT = np.ascontiguousarray(feat_n.T.astype(np.float32)) * FP8_PRESCALE
    featnT8 = _to_dr_layout(featnT, B)

    in_maps = []
    for i in range(NCORES):
        sh = wn[i * CSH:(i + 1) * CSH]  # [CSH, D]
        wt = np.zeros((D, CPAD), dtype=np.float32)
        wt[:, :CSH] = sh.T * FP8_PRESCALE
        in_maps.append(
            {"wnT": _to_dr_layout(wt, CPAD), "featnT": featnT8}
        )

    nc = _build()
    res = run_bass_kernel_spmd(nc, in_maps, list(range(NCORES)))
    _last_results = res

    S = np.zeros(B, dtype=np.float64)
    for i in range(NCORES):
        sc = np.asarray(res.results[i]["out"], dtype=np.float64)
        for bc in range(NBC):
            S[bc * 128:(bc + 1) * 128] += sc[:, bc, :].sum(axis=1)

    Z = S + corr
    per = SHIFT + np.log(Z) - t_logit
    loss = float((coef * per).sum())
    return np.array(loss, dtype=np.float32)


# revision 5
# speedup vs baseline: 1.0382x; 1.0382x over previous
"""AdaFace loss on 8 TRN2 NeuronCores, class-parallel.

Strategy: shard the 100k weight rows (classes) across 8 cores. Host
pre-normalizes rows, transposes to [D, C_shard], scales by 8 and casts to
fp8e4; device computes 64*cos via fp8 DoubleRow matmuls and the
softmax denominator sum(exp(32cos-32)) per batch row. Fixed shift 32
replaces the row max (|logit|<=32), so no collective. Host does the
O(B) combine: margin-target correction, ln, weighted dot.

Device pipeline (v3): PSUM is the choke point (only ACT and DVE can
read it, ~1.1-1.15 ns/col each). PSUM is split into 4 independent
1024-col slots (2 banks each, bufs=1 tags), tiles strictly alternate
ACT/DVE so each engine owns a double-buffered pair of slots and the
PE's production (474ns/tile) hides under consumption (1.2-1.4us/tile):
 - ACT: exp activation + fused accum_out row-sum per tile.
 - DVE: Schraudolph fake-exp (affine to i16 bits = bf16(exp)) into a
   6-plane fi tile; Pool folds planes with two 2048-wide bf16 adds;
   DVE does one 2048-wide row-reduce per batch chunk.
All partial sums land in one [128, NBC, NSLOT] f32 tile, DMA'd out
once; the host sums slots and finishes the loss.
"""

import numpy as np
import ml_dtypes

import concourse.bass as bass
import concourse.tile as tile
from concourse import bacc, mybir
from concourse.bass_utils import run_bass_kernel_spmd

B = 512
D = 256
C = 100000
NCORES = 8
CSH = C // NCORES          # 12500 classes per core
CPAD = 12544               # 12*1024 + 256
NPAD_TOT = (CPAD - CSH) * NCORES

M0 = 0.5
M_MIN = 0.25
SCALE = 32.0
SHIFT = 32.0
FP8_PRESCALE = 8.0         # both operands scaled by 8 -> matmul gives 64*cos

LOG2E = 1.4426950408889634
FA = 64.0 * LOG2E
FB = 16256.0 - 4096.0 * LOG2E

f32 = mybir.dt.float32
bf16 = mybir.dt.bfloat16
i16 = mybir.dt.int16
fp8 = mybir.dt.float8e4

NBC = B // 128             # 4 batch chunks
NSLOT = 10                 # 7 ACT accum slots (6 wide + small) + 2 DVE

_cached_nc = None
_last_results = None


def _schraudolph_rho(fb):
    t = np.linspace(-60.0, -1.0, 200001)
    x = (t + 32.0) * 2.0
    y = np.float32(x) * np.float32(FA) + np.float32(fb)
    i = np.rint(y).astype(np.int16)
    v = i.view(ml_dtypes.bfloat16).astype(np.float64)
    return float(np.mean(v / np.exp(t)))


FB_EFF = FB - 128.0 * np.log2(_schraudolph_rho(FB))
FB_EFF = FB_EFF - 128.0 * np.log2(_schraudolph_rho(FB_EFF))


def _build():
    global _cached_nc
    if _cached_nc is not None:
        return _cached_nc

    nc = bacc.Bacc(
        "TRN2", target_bir_lowering=False, debug=False, num_devices=NCORES
    )

    wnT_d = nc.dram_tensor("wnT", [128, 2, CPAD], fp8, kind="ExternalInput")
    featnT_d = nc.dram_tensor("featnT", [128, 2, B], fp8, kind="ExternalInput")
    out_d = nc.dram_tensor("out", [128, NBC, NSLOT], f32, kind="ExternalOutput")

    with tile.TileContext(nc) as tc:
        with (
            tc.tile_pool(name="persist", bufs=1) as persist,
            tc.tile_pool(name="work", bufs=2) as work,
            tc.tile_pool(name="psum", bufs=1, space="PSUM") as psum,
        ):
            fsb = persist.tile([128, 2, B], fp8)
            nc.sync.dma_start(out=fsb[:], in_=featnT_d[:])

            wsb = persist.tile([128, 2, CPAD], fp8)
            # weight loads in consumption order, 2 HWDGE queues; fine
            # chunks so the PE is never waiting on a late bulk transfer
            plan = [
                (nc.scalar, 0, 1536),
                (nc.sync, 1536, 3072),
                (nc.scalar, 3072, 4608),
                (nc.sync, 4608, 6144),
                (nc.scalar, 6144, 7680),
                (nc.sync, 7680, 9216),
                (nc.scalar, 9216, 10752),
                (nc.sync, 10752, 12544),
            ]
            for eng, lo, hi in plan:
                eng.dma_start(out=wsb[:, :, lo:hi], in_=wnT_d[:, :, lo:hi])

            bias_s = persist.tile([128, 1], f32)
            nc.gpsimd.memset(bias_s[:], -SHIFT)

            S_out = persist.tile([128, NBC, NSLOT], f32)
            nc.gpsimd.memset(S_out[:], 0.0)

            esc = persist.tile([128, 1024], bf16)   # ACT dead-store target

            # per-chunk DVE tile counts: chunk 2 runs 5 DVE tiles to
            # rebalance ACT vs DVE engine totals
            nd_list = [6, 6, 5, 6]
            pend = []   # deferred (tacc, fi6, bc, nd) reduce work
            taccs = {}

            def flush_pending():
                # chunk-k reduce, deferred so it never HOL-blocks the
                # next chunk's affines behind the slow Pool folds
                tacc_p, fi6_p, bc_p, nd_p = pend.pop(0)
                if nd_p == 5:
                    nc.vector.tensor_add(
                        tacc_p[:, 0, :], tacc_p[:, 0, :],
                        fi6_p[:, 4, :].bitcast(bf16),
                    )
                nc.vector.tensor_reduce(
                    S_out[:, bc_p, 8:10],
                    tacc_p[:],
                    axis=mybir.AxisListType.X,
                    op=mybir.AluOpType.add,
                )

            for bc in range(NBC):
                nd = nd_list[bc]
                last = bc == NBC - 1
                lhs = fsb[:, :, bc * 128:(bc + 1) * 128]
                fi6 = work.tile([128, 6, 1024], i16, tag="fi6")
                tacc = work.tile([128, 2, 1024], bf16, tag="tacc", bufs=3)
                a_slot = 0
                d_cnt = 0
                for ti in range(12):
                    c0 = ti * 1024
                    is_dve = (ti % 2 == 1) and d_cnt < nd
                    ps = psum.tile([128, 1024], f32, tag=f"p{ti % 4}")
                    for j in (0, 512):
                        nc.tensor.matmul(
                            ps[:, j:j + 512],
                            lhs,
                            wsb[:, :, c0 + j:c0 + j + 512],
                            start=True, stop=True,
                            perf_mode=mybir.MatmulPerfMode.DoubleRow,
                        )
                    if not is_dve:
                        nc.scalar.activation(
                            esc[:], ps[:],
                            mybir.ActivationFunctionType.Exp,
                            bias=bias_s[:], scale=SCALE / (FP8_PRESCALE**2),
                            accum_out=S_out[:, bc, a_slot:a_slot + 1],
                        )
                        a_slot += 1
                    else:
                        nc.vector.tensor_scalar(
                            fi6[:, d_cnt, :], ps[:],
                            FA, FB_EFF,
                            mybir.AluOpType.mult, mybir.AluOpType.add,
                        )
                        d_cnt += 1
                        if d_cnt == 4:
                            # planes 0|1 + planes 2|3, 2048 wide, on Pool
                            nc.gpsimd.tensor_add(
                                tacc[:],
                                fi6[:, 0:2, :].bitcast(bf16),
                                fi6[:, 2:4, :].bitcast(bf16),
                            )
                        elif d_cnt == 6:
                            if last:
                                # fold 4|5 on DVE: cheap serial endgame
                                nc.vector.tensor_add(
                                    tacc[:], tacc[:],
                                    fi6[:, 4:6, :].bitcast(bf16),
                                )
                            else:
                                nc.gpsimd.tensor_add(
                                    tacc[:], tacc[:],
                                    fi6[:, 4:6, :].bitcast(bf16),
                                )

                # small 256-col tail tile -> ACT
                ps = psum.tile([128, 1024], f32, tag=f"p{12 % 4}")
                nc.tensor.matmul(
                    ps[:, 0:256],
                    lhs,
                    wsb[:, :, 12288:12544],
                    start=True, stop=True,
                    perf_mode=mybir.MatmulPerfMode.DoubleRow,
                )
                nc.scalar.activation(
                    esc[:, 0:256], ps[:, 0:256],
                    mybir.ActivationFunctionType.Exp,
                    bias=bias_s[:], scale=SCALE / (FP8_PRESCALE**2),
                    accum_out=S_out[:, bc, a_slot:a_slot + 1],
                )

                pend.append((tacc, fi6, bc, nd))
                if not last:
                    pass
                if bc >= 1:
                    flush_pending()
            while pend:
                flush_pending()

            nc.sync.dma_start(out=out_d[:], in_=S_out[:])

    nc.compile()
    _cached_nc = nc
    return nc


def _host_prep(features, weight, weights, labels):
    f = features.astype(np.float64)
    norms = np.sqrt((f * f).sum(axis=1))
    lo, hi = norms.min(), norms.max()
    denom = max(hi - lo, 1e-8)
    margins = np.clip(M_MIN + (M0 - M_MIN) * (norms - lo) / denom, M_MIN, M0)
    feat_n = f / np.maximum(norms, 1e-12)[:, None]

    wlab = weight[labels].astype(np.float64)
    wlab_n = wlab / np.maximum(
        np.sqrt((wlab * wlab).sum(axis=1)), 1e-12
    )[:, None]
    cos_t = np.clip((feat_n * wlab_n).sum(axis=1), -1.0 + 1e-7, 1.0 - 1e-7)
    cos_m = cos_t * np.cos(margins) - np.sqrt(1.0 - cos_t * cos_t) * np.sin(
        margins
    )
    t_logit = SCALE * cos_m
    corr = (
        np.exp(SCALE * cos_m - SHIFT)
        - np.exp(SCALE * cos_t - SHIFT)
        - NPAD_TOT * np.exp(-SHIFT)
    )
    coef = weights.astype(np.float64) / B
    return feat_n, corr, coef, t_logit


def _to_dr_layout(mat_t, width):
    """[D, X] f32 -> [128, 2, X] fp8 with k = j*128 + p."""
    a = mat_t.reshape(2, 128, width)          # [j, p, X]
    a = np.ascontiguousarray(a.transpose(1, 0, 2))  # [p, j, X]
    return a.astype(ml_dtypes.float8_e4m3)


def kernel(features, weight, weights, labels):
    global _last_results
    features = np.asarray(features, dtype=np.float32)
    weight = np.asarray(weight, dtype=np.float32)
    weights = np.asarray(weights, dtype=np.float32)
    labels = np.asarray(labels).astype(np.int64)

    feat_n, corr, coef, t_logit = _host_prep(features, weight, weights, labels)

    wn = weight / np.maximum(
        np.linalg.norm(weight, axis=1, keepdims=True), 1e-12
    )
    featnT = np.ascontiguousarray(feat_n.T.astype(np.float32)) * FP8_PRESCALE
    featnT8 = _to_dr_layout(featnT, B)

    in_maps = []
    for i in range(NCORES):
        sh = wn[i * CSH:(i + 1) * CSH]  # [CSH, D]
        wt = np.zeros((D, CPAD), dtype=np.float32)
        wt[:, :CSH] = sh.T * FP8_PRESCALE
        in_maps.append(
            {"wnT": _to_dr_layout(wt, CPAD), "featnT": featnT8}
        )

    nc = _build()
    res = run_bass_kernel_spmd(nc, in_maps, list(range(NCORES)))
    _last_results = res

    S = np.zeros(B, dtype=np.float64)
    for i in range(NCORES):
        sc = np.asarray(res.results[i]["out"], dtype=np.float64)
        for bc in range(NBC):
            S[bc * 128:(bc + 1) * 128] += sc[:, bc, :].sum(axis=1)

    Z = S + corr
    per = SHIFT + np.log(Z) - t_logit
    loss = float((coef * per).sum())
    return np.array(loss, dtype=np.float32)


# revision 6
# speedup vs baseline: 1.2654x; 1.2189x over previous
"""AdaFace loss on 8 TRN2 NeuronCores, class-parallel.

Strategy: shard the 100k weight rows (classes) across 8 cores. Host
pre-normalizes rows, transposes to [D, C_shard], scales by 8 and casts to
fp8e4; device computes 64*cos via fp8 DoubleRow matmuls and the
softmax denominator sum(exp(32cos-32)) per batch row. Fixed shift 32
replaces the row max (|logit|<=32), so no collective. Host does the
O(B) combine: margin-target correction, ln, weighted dot.

Device pipeline (v3): PSUM is the choke point (only ACT and DVE can
read it, ~1.1-1.15 ns/col each). PSUM is split into 4 independent
1024-col slots (2 banks each, bufs=1 tags), tiles strictly alternate
ACT/DVE so each engine owns a double-buffered pair of slots and the
PE's production (474ns/tile) hides under consumption (1.2-1.4us/tile):
 - ACT: exp activation + fused accum_out row-sum per tile.
 - DVE: Schraudolph fake-exp (affine to i16 bits = bf16(exp)) into a
   6-plane fi tile; Pool folds planes with two 2048-wide bf16 adds;
   DVE does one 2048-wide row-reduce per batch chunk.
All partial sums land in one [128, NBC, NSLOT] f32 tile, DMA'd out
once; the host sums slots and finishes the loss.
"""

import numpy as np
import ml_dtypes

import concourse.bass as bass
import concourse.tile as tile
from concourse import bacc, mybir
from concourse.bass_utils import run_bass_kernel_spmd

B = 512
D = 256
C = 100000
NCORES = 8
CSH = C // NCORES          # 12500 classes per core
CPAD = 12544               # 12*1024 + 256
NPAD_TOT = (CPAD - CSH) * NCORES

M0 = 0.5
M_MIN = 0.25
SCALE = 32.0
SHIFT = 32.0
FP8_PRESCALE = 8.0         # both operands scaled by 8 -> matmul gives 64*cos

LOG2E = 1.4426950408889634
FA = 64.0 * LOG2E
FB = 16256.0 - 4096.0 * LOG2E

f32 = mybir.dt.float32
bf16 = mybir.dt.bfloat16
i16 = mybir.dt.int16
fp8 = mybir.dt.float8e4

NBC = B // 128             # 4 batch chunks
NSLOT = 10                 # 7 ACT accum slots (6 wide + small) + 2 DVE

_cached_nc = None
_last_results = None


def _schraudolph_rho(fb):
    t = np.linspace(-60.0, -1.0, 200001)
    x = (t + 32.0) * 2.0
    y = np.float32(x) * np.float32(FA) + np.float32(fb)
    i = np.rint(y).astype(np.int16)
    v = i.view(ml_dtypes.bfloat16).astype(np.float64)
    return float(np.mean(v / np.exp(t)))


FB_EFF = FB - 128.0 * np.log2(_schraudolph_rho(FB))
FB_EFF = FB_EFF - 128.0 * np.log2(_schraudolph_rho(FB_EFF))


def _build():
    global _cached_nc
    if _cached_nc is not None:
        return _cached_nc

    nc = bacc.Bacc(
        "TRN2", target_bir_lowering=False, debug=False, num_devices=NCORES
    )

    wnT_d = nc.dram_tensor("wnT", [128, 2, CPAD], fp8, kind="ExternalInput")
    featnT_d = nc.dram_tensor("featnT", [128, 2, B], fp8, kind="ExternalInput")
    out_d = nc.dram_tensor("out", [128, NBC, NSLOT], f32, kind="ExternalOutput")

    with tile.TileContext(nc) as tc:
        with (
            tc.tile_pool(name="persist", bufs=1) as persist,
            tc.tile_pool(name="work", bufs=2) as work,
            tc.tile_pool(name="psum", bufs=1, space="PSUM") as psum,
        ):
            fsb = persist.tile([128, 2, B], fp8)
            nc.sync.dma_start(out=fsb[:], in_=featnT_d[:])

            wsb = persist.tile([128, 2, CPAD], fp8)
            # weight loads in consumption order, 2 HWDGE queues; fine
            # chunks so the PE is never waiting on a late bulk transfer
            plan = [
                (nc.scalar, 0, 1536),
                (nc.sync, 1536, 3072),
                (nc.scalar, 3072, 4608),
                (nc.sync, 4608, 6144),
                (nc.scalar, 6144, 7680),
                (nc.sync, 7680, 9216),
                (nc.scalar, 9216, 10752),
                (nc.sync, 10752, 12544),
            ]
            for eng, lo, hi in plan:
                eng.dma_start(out=wsb[:, :, lo:hi], in_=wnT_d[:, :, lo:hi])

            bias_s = persist.tile([128, 1], f32)
            nc.gpsimd.memset(bias_s[:], -SHIFT)

            S_out = persist.tile([128, NBC, NSLOT], f32)
            nc.gpsimd.memset(S_out[:], 0.0)

            esc = persist.tile([128, 1024], bf16)   # ACT dead-store target

            # per-chunk DVE tile counts: chunk 2 runs 5 DVE tiles to
            # rebalance ACT vs DVE engine totals
            nd_list = [6, 6, 6, 5]
            pend = []   # deferred (tacc, fi6, bc, nd) reduce work
            taccs = {}

            def flush_pending():
                # chunk-k reduce, deferred so it never HOL-blocks the
                # next chunk's affines behind the slow Pool folds
                tacc_p, fi6_p, bc_p, nd_p = pend.pop(0)
                if nd_p == 5:
                    nc.vector.tensor_add(
                        tacc_p[:, 0, :], tacc_p[:, 0, :],
                        fi6_p[:, 4, :].bitcast(bf16),
                    )
                nc.vector.tensor_reduce(
                    S_out[:, bc_p, 8:10],
                    tacc_p[:],
                    axis=mybir.AxisListType.X,
                    op=mybir.AluOpType.add,
                )

            for bc in range(NBC):
                nd = nd_list[bc]
                last = bc == NBC - 1
                lhs = fsb[:, :, bc * 128:(bc + 1) * 128]
                fi6 = work.tile([128, 6, 1024], i16, tag="fi6")
                tacc = work.tile([128, 2, 1024], bf16, tag="tacc", bufs=3)
                a_slot = 0
                d_cnt = 0
                for ti in range(12):
                    c0 = ti * 1024
                    is_dve = (ti % 2 == 1) and d_cnt < nd
                    ps = psum.tile([128, 1024], f32, tag=f"p{ti % 4}")
                    for j in (0, 512):
                        nc.tensor.matmul(
                            ps[:, j:j + 512],
                            lhs,
                            wsb[:, :, c0 + j:c0 + j + 512],
                            start=True, stop=True,
                            perf_mode=mybir.MatmulPerfMode.DoubleRow,
                        )
                    if not is_dve:
                        nc.scalar.activation(
                            esc[:], ps[:],
                            mybir.ActivationFunctionType.Exp,
                            bias=bias_s[:], scale=SCALE / (FP8_PRESCALE**2),
                            accum_out=S_out[:, bc, a_slot:a_slot + 1],
                        )
                        a_slot += 1
                    else:
                        nc.vector.tensor_scalar(
                            fi6[:, d_cnt, :], ps[:],
                            FA, FB_EFF,
                            mybir.AluOpType.mult, mybir.AluOpType.add,
                        )
                        d_cnt += 1
                        if d_cnt == 4:
                            # planes 0|1 + planes 2|3, 2048 wide, on Pool
                            nc.gpsimd.tensor_add(
                                tacc[:],
                                fi6[:, 0:2, :].bitcast(bf16),
                                fi6[:, 2:4, :].bitcast(bf16),
                            )
                        elif d_cnt == 6:
                            if last:
                                # fold 4|5 on DVE: cheap serial endgame
                                nc.vector.tensor_add(
                                    tacc[:], tacc[:],
                                    fi6[:, 4:6, :].bitcast(bf16),
                                )
                            else:
                                nc.gpsimd.tensor_add(
                                    tacc[:], tacc[:],
                                    fi6[:, 4:6, :].bitcast(bf16),
                                )

                # small 256-col tail tile -> ACT
                ps = psum.tile([128, 1024], f32, tag=f"p{12 % 4}")
                nc.tensor.matmul(
                    ps[:, 0:256],
                    lhs,
                    wsb[:, :, 12288:12544],
                    start=True, stop=True,
                    perf_mode=mybir.MatmulPerfMode.DoubleRow,
                )
                nc.scalar.activation(
                    esc[:, 0:256], ps[:, 0:256],
                    mybir.ActivationFunctionType.Exp,
                    bias=bias_s[:], scale=SCALE / (FP8_PRESCALE**2),
                    accum_out=S_out[:, bc, a_slot:a_slot + 1],
                )

                pend.append((tacc, fi6, bc, nd))
                if not last:
                    pass
                if bc >= 1:
                    flush_pending()
            while pend:
                flush_pending()

            nc.sync.dma_start(out=out_d[:], in_=S_out[:])

    nc.compile()
    _cached_nc = nc
    return nc


def _host_prep(features, weight, weights, labels):
    f = features.astype(np.float64)
    norms = np.sqrt((f * f).sum(axis=1))
    lo, hi = norms.min(), norms.max()
    denom = max(hi - lo, 1e-8)
    margins = np.clip(M_MIN + (M0 - M_MIN) * (norms - lo) / denom, M_MIN, M0)
    feat_n = f / np.maximum(norms, 1e-12)[:, None]

    wlab = weight[labels].astype(np.float64)
    wlab_n = wlab / np.maximum(
        np.sqrt((wlab * wlab).sum(axis=1)), 1e-12
    )[:, None]
    cos_t = np.clip((feat_n * wlab_n).sum(axis=1), -1.0 + 1e-7, 1.0 - 1e-7)
    cos_m = cos_t * np.cos(margins) - np.sqrt(1.0 - cos_t * cos_t) * np.sin(
        margins
    )
    t_logit = SCALE * cos_m
    corr = (
        np.exp(SCALE * cos_m - SHIFT)
        - np.exp(SCALE * cos_t - SHIFT)
        - NPAD_TOT * np.exp(-SHIFT)
    )
    coef = weights.astype(np.float64) / B
    return feat_n, corr, coef, t_logit


def _to_dr_layout(mat_t, width):
    """[D, X] f32 -> [128, 2, X] fp8 with k = j*128 + p."""
    a = mat_t.reshape(2, 128, width)          # [j, p, X]
    a = np.ascontiguousarray(a.transpose(1, 0, 2))  # [p, j, X]
    return a.astype(ml_dtypes.float8_e4m3)


def kernel(features, weight, weights, labels):
    global _last_results
    features = np.asarray(features, dtype=np.float32)
    weight = np.asarray(weight, dtype=np.float32)
    weights = np.asarray(weights, dtype=np.float32)
    labels = np.asarray(labels).astype(np.int64)

    feat_n, corr, coef, t_logit = _host_prep(features, weight, weights, labels)

    wn = weight / np.maximum(
        np.linalg.norm(weight, axis=1, keepdims=True), 1e-12
    )
    featnT = np.ascontiguousarray(feat_n.T.astype(np.float32)) * FP8_PRESCALE
    featnT8 = _to_dr_layout(featnT, B)

    in_maps = []
    for i in range(NCORES):
        sh = wn[i * CSH:(i + 1) * CSH]  # [CSH, D]
        wt = np.zeros((D, CPAD), dtype=np.float32)
        wt[:, :CSH] = sh.T * FP8_PRESCALE
        in_maps.append(
            {"wnT": _to_dr_layout(wt, CPAD), "featnT": featnT8}
        )

    nc = _build()
    res = run_bass_kernel_spmd(nc, in_maps, list(range(NCORES)))
    _last_results = res

    S = np.zeros(B, dtype=np.float64)
    for i in range(NCORES):
        sc = np.asarray(res.results[i]["out"], dtype=np.float64)
        for bc in range(NBC):
            S[bc * 128:(bc + 1) * 128] += sc[:, bc, :].sum(axis=1)

    Z = S + corr
    per = SHIFT + np.log(Z) - t_logit
    loss = float((coef * per).sum())
    return np.array(loss, dtype=np.float32)
